# revision 5
# baseline (speedup 1.0000x reference)
"""Edge-parallel GNN u_mul_v kernel for Trainium2 (8 NeuronCores).

z[e, :] = h[src[e], :] * h[dst[e], :]

Strategy: shard edges across 8 cores (100K each); h (12.8MB) replicated in
HBM as the gather table. The gather primitive is the custom SWDGE
InstDMAGatherAnt (nc.gpsimd.dma_gather): thousands of 256B rows per
instruction, but signed-int16 indices (< 32768). h is therefore addressed as
two tables (h[:32768], h[32768:]) and each core's edges are bucketed on the
host into 4 groups by (src-table, dst-table); the device processes edges in
bucketed order and the host applies the inverse permutation when unsharding
(the edge->slot assignment is part of the sharding).

Per 8192-edge tile: two dma_gathers (src on SWDGE queue 0, dst on queue 1),
one DVE multiply (in place), one contiguous HWDGE store.
"""

import numpy as np

N_NODES = 50000
N_EDGES = 800000
D = 64
N_CORES = 8
E_PER_CORE = N_EDGES // N_CORES  # 100000
L = 32768  # int16-addressable rows per gather table
NI = 8192  # edges per tile (per dma_gather call)
G = NI // 128

_cached = {}  # n_tiles_per_group -> compiled nc


def _build(tiles):
    """tiles: list of (src_hi, dst_hi) per 8192-edge tile."""
    import concourse.bass as bass
    import concourse.tile as tile
    from concourse import bacc, mybir

    T = len(tiles)
    nc = bacc.Bacc(
        "TRN2",
        target_bir_lowering=False,
        debug=False,
        num_devices=N_CORES,
        num_swdge_queues=2,
    )
    h_ap = nc.dram_tensor("h", [N_NODES, D], mybir.dt.float32, kind="ExternalInput").ap()
    si_ap = nc.dram_tensor(
        "src_idx", [T, 128, NI // 16], mybir.dt.int16, kind="ExternalInput"
    ).ap()
    di_ap = nc.dram_tensor(
        "dst_idx", [T, 128, NI // 16], mybir.dt.int16, kind="ExternalInput"
    ).ap()
    z_ap = nc.dram_tensor(
        "z", [T * NI, D], mybir.dt.float32, kind="ExternalOutput"
    ).ap()
    # device z row (t*NI + p*G + g) = gathered position g*128+p of tile t;
    # store is contiguous per partition (G*256B runs).
    z_t = z_ap.rearrange("(t p gd) d -> t p (gd d)", p=128, gd=G)

    tab = {0: h_ap[0:L, :], 1: h_ap[L:N_NODES, :]}

    with tile.TileContext(nc) as tc:
        with (
            tc.tile_pool(name="ix", bufs=4) as ixp,
            tc.tile_pool(name="ga", bufs=3) as gap,
            tc.tile_pool(name="gb", bufs=3) as gbp,
        ):
            for t, (s_hi, d_hi) in enumerate(tiles):
                six = ixp.tile([128, NI // 16], mybir.dt.int16, tag="six")
                nc.sync.dma_start(six[:], si_ap[t])
                dix = ixp.tile([128, NI // 16], mybir.dt.int16, tag="dix")
                nc.sync.dma_start(dix[:], di_ap[t])
                ga = gap.tile([128, G, D], mybir.dt.float32)
                nc.gpsimd.dma_gather(
                    out_ap=ga[:],
                    in_ap=tab[s_hi],
                    idxs_ap=six[:],
                    num_idxs=NI,
                    num_idxs_reg=NI,
                    elem_size=D,
                    single_packet=False,
                    queue_num=0,
                )
                gb = gbp.tile([128, G, D], mybir.dt.float32)
                nc.gpsimd.dma_gather(
                    out_ap=gb[:],
                    in_ap=tab[d_hi],
                    idxs_ap=dix[:],
                    num_idxs=NI,
                    num_idxs_reg=NI,
                    elem_size=D,
                    single_packet=False,
                    queue_num=1,
                )
                nc.vector.tensor_mul(ga[:], ga[:], gb[:])
                nc.sync.dma_start(z_t[t], ga[:])
    nc.compile()
    return nc


def _wrap16(a):
    """[NI] int16 gather-sequence -> wrapped [128, NI//16] layout:
    position i lives at partition i%16, slot i//16, replicated x8."""
    w = a.reshape(NI // 16, 16).T
    return np.ascontiguousarray(np.tile(w, (8, 1)))


# dev z slot p*G+g within a tile holds gathered position g*128+p
_T128 = np.arange(NI).reshape(G, 128).T.reshape(-1)


def _prepare(src, dst):
    """Bucket each core's edges; build per-core packed int16 index tensors,
    the shared tile structure, and the device-order -> original-edge map."""
    src = np.asarray(src).astype(np.int64)
    dst = np.asarray(dst).astype(np.int64)
    groups = []  # [core][g] -> original edge indices (global)
    for c in range(N_CORES):
        lo, hi = c * E_PER_CORE, (c + 1) * E_PER_CORE
        s, d = src[lo:hi], dst[lo:hi]
        g = (s >= L).astype(np.int64) * 2 + (d >= L).astype(np.int64)
        groups.append([np.where(g == k)[0] + lo for k in range(4)])
    caps = [
        -(-max(len(groups[c][k]) for c in range(N_CORES)) // NI) * NI for k in range(4)
    ]
    tiles = []
    for k in range(4):
        tiles += [(k >> 1, k & 1)] * (caps[k] // NI)
    T = len(tiles)

    in_maps = []
    dev_orig = np.empty((N_CORES, T * NI), np.int64)
    for c in range(N_CORES):
        orig = np.full(T * NI, -1, np.int64)
        pos = 0
        for k in range(4):
            e = groups[c][k]
            orig[pos : pos + len(e)] = e
            pos += caps[k]
        s_loc = src[np.maximum(orig, 0)]
        d_loc = dst[np.maximum(orig, 0)]
        # subtract table base, then force padded slots to a valid local 0
        tile_of = np.repeat(np.arange(T), NI)
        s_hi = np.array([t[0] for t in tiles])[tile_of]
        d_hi = np.array([t[1] for t in tiles])[tile_of]
        s16 = np.where(orig >= 0, s_loc - s_hi * L, 0).astype(np.int16)
        d16 = np.where(orig >= 0, d_loc - d_hi * L, 0).astype(np.int16)
        si = np.stack([_wrap16(s16[t * NI : (t + 1) * NI]) for t in range(T)])
        di = np.stack([_wrap16(d16[t * NI : (t + 1) * NI]) for t in range(T)])
        in_maps.append({"si": si, "di": di})
        # device slot order: block t slot p*G+g holds orig[t*NI + g*128+p]
        do = orig.reshape(T, NI)[:, _T128].reshape(-1)
        dev_orig[c] = do
    return tiles, in_maps, dev_orig


def _get_nc(tiles):
    key = tuple(tiles)
    if key not in _cached:
        _cached[key] = _build(list(key))
    return _cached[key]


def _make_in_maps(h, src, dst):
    tiles, idx_maps, dev_orig = _prepare(src, dst)
    h32 = np.ascontiguousarray(h, dtype=np.float32)
    in_maps = [
        {"h": h32, "src_idx": m["si"], "dst_idx": m["di"]} for m in idx_maps
    ]
    return tiles, in_maps, dev_orig


def kernel(h, src, dst):
    from concourse import bass_utils

    tiles, in_maps, dev_orig = _make_in_maps(h, src, dst)
    nc = _get_nc(tiles)
    res = bass_utils.run_bass_kernel_spmd(nc, in_maps, list(range(N_CORES)))
    out = np.empty((N_EDGES, D), np.float32)
    for c in range(N_CORES):
        zc = res.results[c]["z"]
        valid = dev_orig[c] >= 0
        out[dev_orig[c][valid]] = zc[valid]
    return out


# revision 8
# speedup vs baseline: 1.5365x; 1.5365x over previous
"""Edge-parallel GNN u_mul_v kernel for Trainium2 (8 NeuronCores).

z[e, :] = h[src[e], :] * h[dst[e], :]

Strategy: shard edges across 8 cores (100K each); h (12.8MB) replicated in
HBM as the gather table. The gather primitive is the custom SWDGE
InstDMAGatherAnt (nc.gpsimd.dma_gather): thousands of 256B rows per
instruction, but signed-int16 indices (< 32768). h is therefore addressed as
two tables (h[:32768], h[32768:]) and each core's edges are bucketed on the
host into 4 groups by (src-table, dst-table); the device processes edges in
bucketed order and the host applies the inverse permutation when unsharding
(the edge->slot assignment is part of the sharding).

Per 8192-edge tile: two dma_gathers (src on SWDGE queue 0, dst on queue 1),
one DVE multiply (in place), one contiguous HWDGE store.
"""

import numpy as np

N_NODES = 50000
N_EDGES = 800000
D = 64
N_CORES = 8
E_PER_CORE = N_EDGES // N_CORES  # 100000
L = 32768  # int16-addressable rows per gather table
NI = 8192  # edges per tile (per dma_gather call)
G = NI // 128

_cached = {}  # n_tiles_per_group -> compiled nc


def _build(tiles):
    """tiles: list of (src_hi, dst_hi, ni) per tile (ni % 128 == 0, <= NI)."""
    import concourse.bass as bass
    import concourse.tile as tile
    from concourse import bacc, mybir

    T = len(tiles)
    E_DEV = sum(t[2] for t in tiles)
    nc = bacc.Bacc(
        "TRN2",
        target_bir_lowering=False,
        debug=False,
        num_devices=N_CORES,
        num_swdge_queues=2,
    )
    h_ap = nc.dram_tensor("h", [N_NODES, D], mybir.dt.float32, kind="ExternalInput").ap()
    si_ap = nc.dram_tensor(
        "src_idx", [T, 128, NI // 16], mybir.dt.int16, kind="ExternalInput"
    ).ap()
    di_ap = nc.dram_tensor(
        "dst_idx", [T, 128, NI // 16], mybir.dt.int16, kind="ExternalInput"
    ).ap()
    z_ap = nc.dram_tensor("z", [E_DEV, D], mybir.dt.float32, kind="ExternalOutput").ap()

    tab = {0: h_ap[0:L, :], 1: h_ap[L:N_NODES, :]}

    with tile.TileContext(nc) as tc:
        with (
            tc.tile_pool(name="ix", bufs=4) as ixp,
            tc.tile_pool(name="ga", bufs=3) as gap,
            tc.tile_pool(name="gb", bufs=3) as gbp,
        ):
            base = 0
            for t, (s_hi, d_hi, ni) in enumerate(tiles):
                g = ni // 128
                six = ixp.tile([128, ni // 16], mybir.dt.int16, tag="six")
                nc.sync.dma_start(six[:], si_ap[t][:, : ni // 16])
                dix = ixp.tile([128, ni // 16], mybir.dt.int16, tag="dix")
                nc.sync.dma_start(dix[:], di_ap[t][:, : ni // 16])
                ga = gap.tile([128, g, D], mybir.dt.float32, tag="ga")
                nc.gpsimd.dma_gather(
                    out_ap=ga[:],
                    in_ap=tab[s_hi],
                    idxs_ap=six[:],
                    num_idxs=ni,
                    num_idxs_reg=ni,
                    elem_size=D,
                    single_packet=False,
                    queue_num=0,
                )
                gb = gbp.tile([128, g, D], mybir.dt.float32, tag="gb")
                nc.gpsimd.dma_gather(
                    out_ap=gb[:],
                    in_ap=tab[d_hi],
                    idxs_ap=dix[:],
                    num_idxs=ni,
                    num_idxs_reg=ni,
                    elem_size=D,
                    single_packet=False,
                    queue_num=1,
                )
                nc.vector.tensor_mul(ga[:], ga[:], gb[:])
                # device z rows [base : base+ni): slot p*g+gg holds gathered
                # position gg*128+p; contiguous per partition (g*256B runs)
                z_view = z_ap[base : base + ni, :].rearrange(
                    "(p gd) d -> p (gd d)", p=128
                )
                nc.sync.dma_start(z_view, ga[:])
                base += ni
    nc.compile()
    return nc


def _wrap16(a):
    """[ni] int16 gather-sequence -> wrapped [128, ni//16] layout:
    position i lives at partition i%16, slot i//16, replicated x8."""
    w = a.reshape(-1, 16).T
    return np.ascontiguousarray(np.tile(w, (8, 1)))


def _prepare(src, dst):
    """Bucket each core's edges by (src-table, dst-table), sort each bucket by
    src (sequential-ish HBM reads for the src gather), build per-core packed
    int16 index tensors, the shared tile structure (with variable tail tiles),
    and the device-order -> original-edge map."""
    src = np.asarray(src).astype(np.int64)
    dst = np.asarray(dst).astype(np.int64)
    groups = []  # [core][k] -> original edge indices (global), src-sorted
    for c in range(N_CORES):
        lo, hi = c * E_PER_CORE, (c + 1) * E_PER_CORE
        s, d = src[lo:hi], dst[lo:hi]
        g = (s >= L).astype(np.int64) * 2 + (d >= L).astype(np.int64)
        glist = []
        for k in range(4):
            e = np.where(g == k)[0]
            e = e[np.argsort(s[e], kind="stable")]
            glist.append(e + lo)
        groups.append(glist)
    caps = [
        -(-max(len(groups[c][k]) for c in range(N_CORES)) // 128) * 128
        for k in range(4)
    ]
    tiles = []
    for k in range(4):
        rem = caps[k]
        while rem > 0:
            ni = min(NI, rem)
            tiles.append((k >> 1, k & 1, ni))
            rem -= ni
    T = len(tiles)
    E_DEV = sum(t[2] for t in tiles)

    tile_bases = np.cumsum([0] + [t[2] for t in tiles])
    in_maps = []
    dev_orig = np.empty((N_CORES, E_DEV), np.int64)
    for c in range(N_CORES):
        orig = np.full(E_DEV, -1, np.int64)
        pos = 0
        for k in range(4):
            e = groups[c][k]
            orig[pos : pos + len(e)] = e
            pos += caps[k]
        s_loc = src[np.maximum(orig, 0)]
        d_loc = dst[np.maximum(orig, 0)]
        si = np.zeros((T, 128, NI // 16), np.int16)
        di = np.zeros((T, 128, NI // 16), np.int16)
        for t, (s_hi, d_hi, ni) in enumerate(tiles):
            b = tile_bases[t]
            s16 = np.where(
                orig[b : b + ni] >= 0, s_loc[b : b + ni] - s_hi * L, 0
            ).astype(np.int16)
            d16 = np.where(
                orig[b : b + ni] >= 0, d_loc[b : b + ni] - d_hi * L, 0
            ).astype(np.int16)
            si[t, :, : ni // 16] = _wrap16(s16)
            di[t, :, : ni // 16] = _wrap16(d16)
            # device slot p*(ni//128)+g holds gathered position g*128+p
            tmap = np.arange(ni).reshape(ni // 128, 128).T.reshape(-1)
            dev_orig[c, b : b + ni] = orig[b : b + ni][tmap]
        in_maps.append({"si": si, "di": di})
    return tiles, in_maps, dev_orig


def _get_nc(tiles):
    key = tuple(tiles)
    if key not in _cached:
        _cached[key] = _build(list(key))
    return _cached[key]


def _make_in_maps(h, src, dst):
    tiles, idx_maps, dev_orig = _prepare(src, dst)
    h32 = np.ascontiguousarray(h, dtype=np.float32)
    in_maps = [
        {"h": h32, "src_idx": m["si"], "dst_idx": m["di"]} for m in idx_maps
    ]
    return tiles, in_maps, dev_orig


def kernel(h, src, dst):
    from concourse import bass_utils

    tiles, in_maps, dev_orig = _make_in_maps(h, src, dst)
    nc = _get_nc(tiles)
    res = bass_utils.run_bass_kernel_spmd(nc, in_maps, list(range(N_CORES)))
    out = np.empty((N_EDGES, D), np.float32)
    for c in range(N_CORES):
        zc = res.results[c]["z"]
        valid = dev_orig[c] >= 0
        out[dev_orig[c][valid]] = zc[valid]
    return out


# revision 9
# speedup vs baseline: 1.5864x; 1.0325x over previous
"""Edge-parallel GNN u_mul_v kernel for Trainium2 (8 NeuronCores).

z[e, :] = h[src[e], :] * h[dst[e], :]

Strategy: shard edges across 8 cores (100K each); h (12.8MB) replicated in
HBM as the gather table. The gather primitive is the custom SWDGE
InstDMAGatherAnt (nc.gpsimd.dma_gather): thousands of 256B rows per
instruction, but signed-int16 indices (< 32768). h is therefore addressed as
two tables (h[:32768], h[32768:]) and each core's edges are bucketed on the
host into 4 groups by (src-table, dst-table); the device processes edges in
bucketed order and the host applies the inverse permutation when unsharding
(the edge->slot assignment is part of the sharding).

Per 8192-edge tile: two dma_gathers (src on SWDGE queue 0, dst on queue 1),
one DVE multiply (in place), one contiguous HWDGE store.
"""

import numpy as np

N_NODES = 50000
N_EDGES = 800000
D = 64
N_CORES = 8
E_PER_CORE = N_EDGES // N_CORES  # 100000
L = 32768  # int16-addressable rows per gather table
NI = 8192  # edges per tile (per dma_gather call)
G = NI // 128

_cached = {}  # n_tiles_per_group -> compiled nc


def _build(tiles):
    """tiles: list of (src_hi, dst_hi, ni) per tile (ni % 128 == 0, <= NI)."""
    import concourse.bass as bass
    import concourse.tile as tile
    from concourse import bacc, mybir

    T = len(tiles)
    E_DEV = sum(t[2] for t in tiles)
    nc = bacc.Bacc(
        "TRN2",
        target_bir_lowering=False,
        debug=False,
        num_devices=N_CORES,
        num_swdge_queues=2,
    )
    h_ap = nc.dram_tensor("h", [N_NODES, D], mybir.dt.float32, kind="ExternalInput").ap()
    si_ap = nc.dram_tensor(
        "src_idx", [T, 128, NI // 16], mybir.dt.int16, kind="ExternalInput"
    ).ap()
    di_ap = nc.dram_tensor(
        "dst_idx", [T, 128, NI // 16], mybir.dt.int16, kind="ExternalInput"
    ).ap()
    z_ap = nc.dram_tensor("z", [E_DEV, D], mybir.dt.float32, kind="ExternalOutput").ap()

    tab = {0: h_ap[0:L, :], 1: h_ap[L:N_NODES, :]}

    with tile.TileContext(nc) as tc:
        with (
            tc.tile_pool(name="ix", bufs=6) as ixp,
            tc.tile_pool(name="ga", bufs=4) as gap,
            tc.tile_pool(name="gb", bufs=4) as gbp,
        ):
            base = 0
            for t, (s_hi, d_hi, ni) in enumerate(tiles):
                g = ni // 128
                six = ixp.tile([128, ni // 16], mybir.dt.int16, tag="six")
                nc.sync.dma_start(six[:], si_ap[t][:, : ni // 16])
                dix = ixp.tile([128, ni // 16], mybir.dt.int16, tag="dix")
                nc.sync.dma_start(dix[:], di_ap[t][:, : ni // 16])
                ga = gap.tile([128, g, D], mybir.dt.float32, tag="ga")
                nc.gpsimd.dma_gather(
                    out_ap=ga[:],
                    in_ap=tab[s_hi],
                    idxs_ap=six[:],
                    num_idxs=ni,
                    num_idxs_reg=ni,
                    elem_size=D,
                    single_packet=False,
                    queue_num=0,
                )
                gb = gbp.tile([128, g, D], mybir.dt.float32, tag="gb")
                nc.gpsimd.dma_gather(
                    out_ap=gb[:],
                    in_ap=tab[d_hi],
                    idxs_ap=dix[:],
                    num_idxs=ni,
                    num_idxs_reg=ni,
                    elem_size=D,
                    single_packet=False,
                    queue_num=1,
                )
                nc.vector.tensor_mul(ga[:], ga[:], gb[:])
                # device z rows [base : base+ni): slot p*g+gg holds gathered
                # position gg*128+p; contiguous per partition (g*256B runs)
                z_view = z_ap[base : base + ni, :].rearrange(
                    "(p gd) d -> p (gd d)", p=128
                )
                nc.sync.dma_start(z_view, ga[:])
                base += ni
    nc.compile()
    return nc


def _wrap16(a):
    """[ni] int16 gather-sequence -> wrapped [128, ni//16] layout:
    position i lives at partition i%16, slot i//16, replicated x8."""
    w = a.reshape(-1, 16).T
    return np.ascontiguousarray(np.tile(w, (8, 1)))


def _prepare(src, dst):
    """Bucket each core's edges by (src-table, dst-table), sort each bucket by
    src (sequential-ish HBM reads for the src gather), build per-core packed
    int16 index tensors, the shared tile structure (with variable tail tiles),
    and the device-order -> original-edge map."""
    src = np.asarray(src).astype(np.int64)
    dst = np.asarray(dst).astype(np.int64)
    groups = []  # [core][k] -> original edge indices (global), src-sorted
    for c in range(N_CORES):
        lo, hi = c * E_PER_CORE, (c + 1) * E_PER_CORE
        s, d = src[lo:hi], dst[lo:hi]
        g = (s >= L).astype(np.int64) * 2 + (d >= L).astype(np.int64)
        glist = []
        for k in range(4):
            e = np.where(g == k)[0]
            e = e[np.argsort(s[e], kind="stable")]
            glist.append(e + lo)
        groups.append(glist)
    caps = [
        -(-max(len(groups[c][k]) for c in range(N_CORES)) // 128) * 128
        for k in range(4)
    ]
    tiles = []
    for k in range(4):
        rem = caps[k]
        while rem > 0:
            ni = min(NI, rem)
            tiles.append((k >> 1, k & 1, ni))
            rem -= ni
    T = len(tiles)
    E_DEV = sum(t[2] for t in tiles)

    tile_bases = np.cumsum([0] + [t[2] for t in tiles])
    in_maps = []
    dev_orig = np.empty((N_CORES, E_DEV), np.int64)
    for c in range(N_CORES):
        orig = np.full(E_DEV, -1, np.int64)
        pos = 0
        for k in range(4):
            e = groups[c][k]
            orig[pos : pos + len(e)] = e
            pos += caps[k]
        s_loc = src[np.maximum(orig, 0)]
        d_loc = dst[np.maximum(orig, 0)]
        si = np.zeros((T, 128, NI // 16), np.int16)
        di = np.zeros((T, 128, NI // 16), np.int16)
        for t, (s_hi, d_hi, ni) in enumerate(tiles):
            b = tile_bases[t]
            s16 = np.where(
                orig[b : b + ni] >= 0, s_loc[b : b + ni] - s_hi * L, 0
            ).astype(np.int16)
            d16 = np.where(
                orig[b : b + ni] >= 0, d_loc[b : b + ni] - d_hi * L, 0
            ).astype(np.int16)
            si[t, :, : ni // 16] = _wrap16(s16)
            di[t, :, : ni // 16] = _wrap16(d16)
            # device slot p*(ni//128)+g holds gathered position g*128+p
            tmap = np.arange(ni).reshape(ni // 128, 128).T.reshape(-1)
            dev_orig[c, b : b + ni] = orig[b : b + ni][tmap]
        in_maps.append({"si": si, "di": di})
    return tiles, in_maps, dev_orig


def _get_nc(tiles):
    key = tuple(tiles)
    if key not in _cached:
        _cached[key] = _build(list(key))
    return _cached[key]


def _make_in_maps(h, src, dst):
    tiles, idx_maps, dev_orig = _prepare(src, dst)
    h32 = np.ascontiguousarray(h, dtype=np.float32)
    in_maps = [
        {"h": h32, "src_idx": m["si"], "dst_idx": m["di"]} for m in idx_maps
    ]
    return tiles, in_maps, dev_orig


def kernel(h, src, dst):
    from concourse import bass_utils

    tiles, in_maps, dev_orig = _make_in_maps(h, src, dst)
    nc = _get_nc(tiles)
    res = bass_utils.run_bass_kernel_spmd(nc, in_maps, list(range(N_CORES)))
    out = np.empty((N_EDGES, D), np.float32)
    for c in range(N_CORES):
        zc = res.results[c]["z"]
        valid = dev_orig[c] >= 0
        out[dev_orig[c][valid]] = zc[valid]
    return out


# revision 10
# speedup vs baseline: 3.0091x; 1.8968x over previous
"""Edge-parallel GNN u_mul_v kernel for Trainium2 (8 NeuronCores).

z[e, :] = h[src[e], :] * h[dst[e], :]

Strategy: shard edges across 8 cores (100K each); h (12.8MB) replicated in
HBM as the gather table. The gather primitive is the custom SWDGE
InstDMAGatherAnt (nc.gpsimd.dma_gather): thousands of 256B rows per
instruction, but signed-int16 indices (< 32768). h is therefore addressed as
two tables (h[:32768], h[32768:]) and each core's edges are bucketed on the
host into 4 groups by (src-table, dst-table); the device processes edges in
bucketed order and the host applies the inverse permutation when unsharding
(the edge->slot assignment is part of the sharding).

Per 8192-edge tile: two dma_gathers (src on SWDGE queue 0, dst on queue 1),
one DVE multiply (in place), one contiguous HWDGE store.
"""

import numpy as np

N_NODES = 50000
N_EDGES = 800000
D = 64
N_CORES = 8
E_PER_CORE = N_EDGES // N_CORES  # 100000
L = 32768  # int16-addressable rows per gather table
NI = 8192  # edges per tile (per dma_gather call)
G = NI // 128

_cached = {}  # n_tiles_per_group -> compiled nc


def _build(tiles):
    """tiles: list of (src_hi, dst_hi, ni) per tile (ni % 128 == 0, <= NI)."""
    import concourse.bass as bass
    import concourse.tile as tile
    from concourse import bacc, mybir

    T = len(tiles)
    E_DEV = sum(t[2] for t in tiles)
    nc = bacc.Bacc(
        "TRN2",
        target_bir_lowering=False,
        debug=False,
        num_devices=N_CORES,
        num_swdge_queues=4,
    )
    h_ap = nc.dram_tensor("h", [N_NODES, D], mybir.dt.float32, kind="ExternalInput").ap()
    si_ap = nc.dram_tensor(
        "src_idx", [T, 128, NI // 16], mybir.dt.int16, kind="ExternalInput"
    ).ap()
    di_ap = nc.dram_tensor(
        "dst_idx", [T, 128, NI // 16], mybir.dt.int16, kind="ExternalInput"
    ).ap()
    z_ap = nc.dram_tensor("z", [E_DEV, D], mybir.dt.float32, kind="ExternalOutput").ap()

    tab = {0: h_ap[0:L, :], 1: h_ap[L:N_NODES, :]}

    with tile.TileContext(nc) as tc:
        with (
            tc.tile_pool(name="ix", bufs=6) as ixp,
            tc.tile_pool(name="ga", bufs=4) as gap,
            tc.tile_pool(name="gb", bufs=4) as gbp,
        ):
            base = 0
            for t, (s_hi, d_hi, ni) in enumerate(tiles):
                g = ni // 128
                six = ixp.tile([128, ni // 16], mybir.dt.int16, tag="six")
                nc.sync.dma_start(six[:], si_ap[t][:, : ni // 16])
                dix = ixp.tile([128, ni // 16], mybir.dt.int16, tag="dix")
                nc.sync.dma_start(dix[:], di_ap[t][:, : ni // 16])
                ga = gap.tile([128, g, D], mybir.dt.float32, tag="ga")
                nc.gpsimd.dma_gather(
                    out_ap=ga[:],
                    in_ap=tab[s_hi],
                    idxs_ap=six[:],
                    num_idxs=ni,
                    num_idxs_reg=ni,
                    elem_size=D,
                    single_packet=False,
                    queue_num=(t % 2) * 2,
                )
                gb = gbp.tile([128, g, D], mybir.dt.float32, tag="gb")
                nc.gpsimd.dma_gather(
                    out_ap=gb[:],
                    in_ap=tab[d_hi],
                    idxs_ap=dix[:],
                    num_idxs=ni,
                    num_idxs_reg=ni,
                    elem_size=D,
                    single_packet=False,
                    queue_num=(t % 2) * 2 + 1,
                )
                nc.vector.tensor_mul(ga[:], ga[:], gb[:])
                # device z rows [base : base+ni): slot p*g+gg holds gathered
                # position gg*128+p; contiguous per partition (g*256B runs)
                z_view = z_ap[base : base + ni, :].rearrange(
                    "(p gd) d -> p (gd d)", p=128
                )
                nc.sync.dma_start(z_view, ga[:])
                base += ni
    nc.compile()
    return nc


def _wrap16(a):
    """[ni] int16 gather-sequence -> wrapped [128, ni//16] layout:
    position i lives at partition i%16, slot i//16, replicated x8."""
    w = a.reshape(-1, 16).T
    return np.ascontiguousarray(np.tile(w, (8, 1)))


def _prepare(src, dst):
    """Bucket each core's edges by (src-table, dst-table), sort each bucket by
    src (sequential-ish HBM reads for the src gather), build per-core packed
    int16 index tensors, the shared tile structure (with variable tail tiles),
    and the device-order -> original-edge map."""
    src = np.asarray(src).astype(np.int64)
    dst = np.asarray(dst).astype(np.int64)
    groups = []  # [core][k] -> original edge indices (global), src-sorted
    for c in range(N_CORES):
        lo, hi = c * E_PER_CORE, (c + 1) * E_PER_CORE
        s, d = src[lo:hi], dst[lo:hi]
        g = (s >= L).astype(np.int64) * 2 + (d >= L).astype(np.int64)
        glist = []
        for k in range(4):
            e = np.where(g == k)[0]
            e = e[np.argsort(s[e], kind="stable")]
            glist.append(e + lo)
        groups.append(glist)
    caps = [
        -(-max(len(groups[c][k]) for c in range(N_CORES)) // 128) * 128
        for k in range(4)
    ]
    tiles = []
    for k in range(4):
        rem = caps[k]
        while rem > 0:
            ni = min(NI, rem)
            tiles.append((k >> 1, k & 1, ni))
            rem -= ni
    T = len(tiles)
    E_DEV = sum(t[2] for t in tiles)

    tile_bases = np.cumsum([0] + [t[2] for t in tiles])
    in_maps = []
    dev_orig = np.empty((N_CORES, E_DEV), np.int64)
    for c in range(N_CORES):
        orig = np.full(E_DEV, -1, np.int64)
        pos = 0
        for k in range(4):
            e = groups[c][k]
            orig[pos : pos + len(e)] = e
            pos += caps[k]
        s_loc = src[np.maximum(orig, 0)]
        d_loc = dst[np.maximum(orig, 0)]
        si = np.zeros((T, 128, NI // 16), np.int16)
        di = np.zeros((T, 128, NI // 16), np.int16)
        for t, (s_hi, d_hi, ni) in enumerate(tiles):
            b = tile_bases[t]
            s16 = np.where(
                orig[b : b + ni] >= 0, s_loc[b : b + ni] - s_hi * L, 0
            ).astype(np.int16)
            d16 = np.where(
                orig[b : b + ni] >= 0, d_loc[b : b + ni] - d_hi * L, 0
            ).astype(np.int16)
            si[t, :, : ni // 16] = _wrap16(s16)
            di[t, :, : ni // 16] = _wrap16(d16)
            # device slot p*(ni//128)+g holds gathered position g*128+p
            tmap = np.arange(ni).reshape(ni // 128, 128).T.reshape(-1)
            dev_orig[c, b : b + ni] = orig[b : b + ni][tmap]
        in_maps.append({"si": si, "di": di})
    return tiles, in_maps, dev_orig


def _get_nc(tiles):
    key = tuple(tiles)
    if key not in _cached:
        _cached[key] = _build(list(key))
    return _cached[key]


def _make_in_maps(h, src, dst):
    tiles, idx_maps, dev_orig = _prepare(src, dst)
    h32 = np.ascontiguousarray(h, dtype=np.float32)
    in_maps = [
        {"h": h32, "src_idx": m["si"], "dst_idx": m["di"]} for m in idx_maps
    ]
    return tiles, in_maps, dev_orig


def kernel(h, src, dst):
    from concourse import bass_utils

    tiles, in_maps, dev_orig = _make_in_maps(h, src, dst)
    nc = _get_nc(tiles)
    res = bass_utils.run_bass_kernel_spmd(nc, in_maps, list(range(N_CORES)))
    out = np.empty((N_EDGES, D), np.float32)
    for c in range(N_CORES):
        zc = res.results[c]["z"]
        valid = dev_orig[c] >= 0
        out[dev_orig[c][valid]] = zc[valid]
    return out
